# revision 7
# baseline (speedup 1.0000x reference)
"""Trainium2 Bass kernel for nn_DataEmbedding_v2 — v4.

Pure data parallel over batch (2 per core).  Output stored f16 on device
(tolerance is rel 2e-2 vs absmax ~1e3; f16 adds ~5e-4) and upcast to f32 on
host.  tau (the tc scan) keeps the exact-f32 compare algebra of v3 (f16
hi/mid/lo splits accumulated coarse-first in f32 psum), but the surrounding
dataflow is rebuilt for engine balance and PE warmth:

  - warmup matmul burst while input DMAs land (HAM un-throttle)
  - select matmuls: diag (K=6 @ rows 96:102) runs row-tile-concurrent with
    the one-hot gather (K=96 @ rows 0:96), one psum bank pair per 4 tiles
  - sign+value masking fused into one DVE scalar_tensor_tensor (is_gt, mult)
  - conv: tap01 (K=128) then tap2 (K=64 @ rows 0:64) concurrent with the
    tau embedding (K=2 @ rows 64:66); pe(+tc_b) injected via identity matmul
    for ACT-drained tiles, fused into the DVE drain otherwise
  - conv matmuls software-pipelined: tap matmuls run ahead of tau (LAG
    tiles), so the PE never stalls on the tau roundtrip
  - all small consts packed into two blob DMAs
"""

import math
import os
import sys

sys.path.insert(0, "/opt/trn_rl_repo")

import numpy as np

B, S, C, D = 16, 2048, 64, 512
NCORES = 8
BLOC = B // NCORES
P = 128
NT = S // P
ETA = 0.3
EPS = 1e-08
BIG = 60000.0

WARM_N = 44          # warmup matmuls
LAG = 3              # conv tiles run ahead of tau
OG = 8               # output tiles per DMA group
# drain engine per conv tile: 'V' = DVE fused (+pe16), 'A' = ACT copy (pe
# injected via matmul)
DRAIN = ["V", "A"] * (NT // 2)

# blob16 column layout
B16_IDENT = 0
B16_DVAL = 128
B16_SVAL = B16_DVAL + NT * P
B16_MASK1 = B16_SVAL + P
B16_W = B16_MASK1 + NT * NT
# blob32 column layout
B32_IDENT = 0
B32_T1C = 128
B32_KCOL = B32_T1C + NT
B32_W = B32_KCOL + 1


def _emit(tc, aps):
    from contextlib import ExitStack

    from concourse import mybir

    f32 = mybir.dt.float32
    f16 = mybir.dt.float16
    Alu = mybir.AluOpType
    Ax = mybir.AxisListType

    nc = tc.nc
    xin, xta, pe16 = aps["xin"], aps["xta"], aps["pe16"]
    wt01, wtbc = aps["wt01"], aps["wtbc"]
    blob16, blob32 = aps["blob16"], aps["blob32"]
    out = aps["out"]
    js_dram, tau_dram, spT_d = aps["js_dram"], aps["tau_dram"], aps["spT_d"]

    with ExitStack() as ctx:
        singles = ctx.enter_context(tc.tile_pool(name="singles", bufs=1))
        xpool = ctx.enter_context(tc.tile_pool(name="xpool", bufs=2))
        xtp = ctx.enter_context(tc.tile_pool(name="xtp", bufs=2))
        small = ctx.enter_context(tc.tile_pool(name="small", bufs=2))
        selp = ctx.enter_context(tc.tile_pool(name="selp", bufs=2))
        sgp = ctx.enter_context(tc.tile_pool(name="sgp", bufs=3))
        outp = ctx.enter_context(tc.tile_pool(name="outp", bufs=2))
        psA = ctx.enter_context(tc.tile_pool(name="psA", bufs=4, space="PSUM"))
        psD = ctx.enter_context(tc.tile_pool(name="psD", bufs=2, space="PSUM"))

        # ---- early DMAs: blobs + x(+xta) so compute can start ASAP ----
        b16 = singles.tile([P, B16_W], f16)
        nc.sync.dma_start(b16, blob16)
        b32 = singles.tile([P, B32_W], f32)
        nc.sync.dma_start(b32, blob32)
        identh = b16[:, B16_IDENT : B16_IDENT + P]
        dval = b16[:, B16_DVAL:B16_SVAL].rearrange("p (i u) -> p i u", i=NT)
        sval = b16[:, B16_SVAL:B16_MASK1]
        mask1 = b16[:, B16_MASK1:B16_W].rearrange("p (i j) -> p i j", i=NT)
        identf = b32[:, B32_IDENT : B32_IDENT + P]
        t1c = b32[:, B32_T1C : B32_T1C + NT]
        kcol96 = b32[:, B32_KCOL : B32_KCOL + 1]

        xins = {}
        xtas = {}
        for b in range(BLOC):
            xin_sb = xpool.tile([P, NT, C], f32, tag="xin", name=f"xin{b}")
            nc.scalar.dma_start(xin_sb, xin[b].rearrange("(j p) c -> p j c", p=P))
            xins[b] = xin_sb
            xta_sb = xtp.tile([P, S + 2], f16, tag="xta", name=f"xta{b}")
            nc.gpsimd.dma_start(xta_sb, xta[b])
            xtas[b] = xta_sb

        # warmup dummy tile (no DMA dependency)
        wdum = singles.tile([P, P], f16)
        nc.gpsimd.memset(wdum, 1.0)

        wt01_sb = singles.tile([P, D], f16)
        nc.sync.dma_start(wt01_sb, wt01)
        wtbc_sb = singles.tile([66, D], f16)
        nc.sync.dma_start(wtbc_sb, wtbc)
        pe_sb = singles.tile([P, NT, D], f16)
        for q in range(4):
            nc.sync.dma_start(
                pe_sb[:, q * 4 : (q + 1) * 4, :],
                pe16[:, q * 4 : (q + 1) * 4, :],
            )

        # ---- PE warmup burst: back-to-back matmuls while DMAs land ----
        for w in range(WARM_N):
            psw = psA.tile([P, P], f32, tag="psa", name=f"warm{w}")
            nc.tensor.matmul(psw, lhsT=wdum, rhs=wdum, start=True, stop=True)

        # ---- phase A per batch ----
        st = {}
        for b in range(BLOC):
            xin_sb = xins[b]
            normc = small.tile([P, NT], f32, tag="normc", name=f"normc{b}")
            nc.vector.tensor_reduce(
                normc, xin_sb, axis=Ax.X, op=Alu.add, apply_absolute_value=True
            )
            negthc = small.tile([P, 2, NT], f32, tag="negthc", name=f"negthc{b}")
            nc.vector.tensor_scalar(negthc[:, 0, :], normc, -1.0, None, op0=Alu.mult)
            nc.vector.tensor_scalar(
                negthc[:, 1, :],
                normc,
                float(EPS),
                float(1.0 - ETA),
                op0=Alu.add,
                op1=Alu.mult,
            )
            # f16 3-way splits (exact): layout [P, {n,q}, k, NT]
            spl = small.tile([P, 2, 3, NT], f16, tag="spl", name=f"spl{b}")
            nc.vector.tensor_copy(spl[:, :, 0, :], negthc)
            rem1 = small.tile([P, 2, NT], f32, tag="rem1", name=f"rem1_{b}")
            nc.vector.tensor_tensor(rem1, negthc, spl[:, :, 0, :], op=Alu.subtract)
            nc.vector.tensor_copy(spl[:, :, 1, :], rem1)
            rem2 = small.tile([P, 2, NT], f32, tag="rem2", name=f"rem2_{b}")
            nc.vector.tensor_tensor(rem2, rem1, spl[:, :, 1, :], op=Alu.subtract)
            nc.vector.tensor_copy(spl[:, :, 2, :], rem2)
            # transpose the six split vectors: spT[(nq)*48+k*16+i, m]
            ps96 = psD.tile([96, P], f16, tag="psd", name=f"ps96_{b}")
            nc.tensor.transpose(ps96, spl.rearrange("p a k j -> p (a k j)"), identh)
            spT = small.tile([96, P], f16, tag="spt", name=f"spT{b}")
            nc.scalar.copy(spT, ps96)
            nc.sync.dma_start(spT_d[b], spT)
            # block minima of norms (as -max of -norms), exact f32
            psnT = psD.tile([NT, P], f32, tag="psd", name=f"psnT{b}")
            nc.tensor.transpose(psnT, negthc[:, 0, :], identf)
            bneg = small.tile([NT, 1], f32, tag="bneg", name=f"bneg{b}")
            nc.vector.tensor_reduce(bneg, psnT, axis=Ax.X, op=Alu.max)
            psb1 = psD.tile([1, NT], f32, tag="psd", name=f"psb1_{b}")
            nc.tensor.transpose(psb1, bneg, identf[0:NT, 0:NT])
            bnegrow = small.tile([1, NT], f32, tag="bnegrow", name=f"bnegrow{b}")
            nc.scalar.copy(bnegrow, psb1)
            bnegbc = small.tile([P, NT], f32, tag="bnegbc", name=f"bnegbc{b}")
            nc.gpsimd.partition_broadcast(bnegbc, bnegrow)
            # level 1: jp1 = 1 + last block j<i with bmin_j < thresh (0 = none)
            l1a = small.tile([P, NT, NT], f32, tag="l1a", name=f"l1a{b}")
            nc.vector.tensor_tensor(
                l1a,
                negthc[:, 1, :].unsqueeze(2).to_broadcast([P, NT, NT]),
                bnegbc.unsqueeze(1).to_broadcast([P, NT, NT]),
                op=Alu.add,
            )
            l1c = small.tile([P, NT, NT], f16, tag="l1c", name=f"l1c{b}")
            nc.vector.scalar_tensor_tensor(
                l1c, l1a, 0.0, mask1, op0=Alu.is_gt, op1=Alu.mult
            )
            jp1 = small.tile([P, NT], f16, tag="jp1", name=f"jp1_{b}")
            nc.vector.tensor_reduce(jp1, l1c, axis=Ax.X, op=Alu.max)
            psj = psD.tile([NT, P], f16, tag="psd", name=f"psj{b}")
            nc.tensor.transpose(psj, jp1, identh)
            jpT = small.tile([NT, P], f16, tag="jpt", name=f"jpT{b}")
            nc.scalar.copy(jpT, psj)
            nc.sync.dma_start(js_dram[b].rearrange("(i p) -> i p", p=P), jpT)
            st[b] = (spl, jp1)

        # ---- select setup per batch: T1 (lhsT) / T2 (rhs) ----
        sel = {}
        for b in range(BLOC):
            # T1 rows 0:96 one-hot (+q rows 17/49/81), rows 96:102 diag lhsT
            t1 = selp.tile([102, S], f16, tag="t1", name=f"t1_{b}")
            ohsrc = selp.tile([96, S], f16, tag="ohsrc", name=f"ohsrc{b}")
            nc.gpsimd.dma_start(ohsrc, js_dram[b].partition_broadcast(96))
            nc.vector.tensor_scalar(
                t1[0:96, :], ohsrc, kcol96[0:96], None, op0=Alu.is_equal
            )
            nc.scalar.dma_start(
                t1[17:82:32, :], spT_d[b, 48:96].rearrange("(k i) m -> k (i m)", k=3)
            )
            nc.gpsimd.dma_start(t1[96:102, :], aps["dtmpl"])
            nc.scalar.dma_start(
                t1[97:102:2, :], spT_d[b, 48:96].rearrange("(k i) m -> k (i m)", k=3)
            )
            # T2 rows 0:96 value tables (cols 0:128), rows 96:102 diag rhs
            t2 = selp.tile([102, S], f16, tag="t2", name=f"t2_{b}")
            nc.gpsimd.dma_start(t2[0:96, 0:P], aps["nbtt"])
            for g in range(3):
                nc.scalar.dma_start(
                    t2[g * 32 : g * 32 + 16, 0:P], spT_d[b, g * 16 : (g + 1) * 16]
                )
            nc.gpsimd.dma_start(t2[96:102, :], aps["rtmpl"])
            nc.scalar.dma_start(
                t2[96:102:2, :], spT_d[b, 0:48].rearrange("(k i) m -> k (i m)", k=3)
            )
            sel[b] = (t1, t2)

        # absorb multi-writer waits once, off the critical path
        for b in range(BLOC):
            t1, t2 = sel[b]
            nc.tensor.ldweights(t1[:, 0:P])
            nc.tensor.ldweights(t2[:, 0:P])

        # ---- select matmuls + masked value extraction per batch ----
        tautiles = {}
        for b in range(BLOC):
            t1, t2 = sel[b]
            rdiag = small.tile([P, NT], f16, tag="rdiag", name=f"rdiag{b}")
            rsel = small.tile([P, NT], f16, tag="rsel", name=f"rsel{b}")
            for h in range(4):
                psd = psD.tile([P, 4, 2, P], f32, tag="psd", name=f"psd{b}_{h}")
                for ii in range(4):
                    i = h * 4 + ii
                    sl = slice(i * P, (i + 1) * P)
                    nc.tensor.matmul(
                        psd[:, ii, 0, :],
                        lhsT=t1[96:102, sl],
                        rhs=t2[96:102, sl],
                        start=True,
                        stop=True,
                        tile_position=(96, 0),
                    )
                    nc.tensor.matmul(
                        psd[:, ii, 1, :],
                        lhsT=t1[0:96, sl],
                        rhs=t2[0:96, 0:P],
                        start=True,
                        stop=True,
                    )
                dm = sgp.tile([P, 4, P], f16, tag="dm", name=f"dm{b}_{h}")
                nc.vector.scalar_tensor_tensor(
                    dm,
                    psd[:, :, 0, :],
                    0.0,
                    dval[:, h * 4 : (h + 1) * 4, :],
                    op0=Alu.is_gt,
                    op1=Alu.mult,
                )
                sm = sgp.tile([P, 4, P], f16, tag="sm", name=f"sm{b}_{h}")
                nc.vector.scalar_tensor_tensor(
                    sm,
                    psd[:, :, 1, :],
                    0.0,
                    sval.unsqueeze(1).to_broadcast([P, 4, P]),
                    op0=Alu.is_gt,
                    op1=Alu.mult,
                )
                nc.vector.tensor_reduce(
                    rdiag[:, h * 4 : (h + 1) * 4], dm, axis=Ax.X, op=Alu.max
                )
                nc.vector.tensor_reduce(
                    rsel[:, h * 4 : (h + 1) * 4], sm, axis=Ax.X, op=Alu.max
                )

            # ans = max(diag, (jp1-1)*128 + rsel); tau = ans>0 ? t+1-ans : 0
            cand2 = small.tile([P, NT], f16, tag="cand2", name=f"cand2_{b}")
            nc.vector.tensor_scalar(
                cand2, st[b][1], 128.0, -128.0, op0=Alu.mult, op1=Alu.add
            )
            cand2b = small.tile([P, NT], f16, tag="cand2b", name=f"cand2b{b}")
            nc.vector.tensor_tensor(cand2b, cand2, rsel, op=Alu.add)
            ans = small.tile([P, NT], f16, tag="ans", name=f"ans{b}")
            nc.vector.tensor_tensor(ans, cand2b, rdiag, op=Alu.max)
            td = small.tile([P, NT], f32, tag="td", name=f"td{b}")
            nc.vector.tensor_tensor(td, t1c, ans, op=Alu.subtract)
            tauc = small.tile([P, NT], f32, tag="tauc", name=f"tauc{b}")
            nc.vector.scalar_tensor_tensor(
                tauc, ans, 0.0, td, op0=Alu.is_gt, op1=Alu.mult
            )
            nc.sync.dma_start(tau_dram[b].rearrange("(p j) -> p j", p=P), tauc)
            tautile = selp.tile([66, S], f16, tag="taut", name=f"taut{b}")
            nc.gpsimd.dma_start(
                tautile[64:66, :], tau_dram[b].partition_broadcast(2)
            )
            tautiles[b] = tautile

        # ---- conv + pe + tau embedding, software-pipelined ----
        for b in range(BLOC):
            xta_sb = xtas[b]
            tautile = tautiles[b]
            taujp = tautile[64:66, :].rearrange("q (p j) -> q j p", j=NT)
            nc.tensor.ldweights(taujp[:, 0, :], tile_position=(64, 0))
            nc.tensor.ldweights(xta_sb[:, 0:P])

            if b == BLOC - 1:
                groups = [(0, OG), (OG, 4), (OG + 4, 2), (OG + 6, 1), (OG + 7, 1)]
            else:
                groups = [(g * OG, OG) for g in range(NT // OG)]

            osbs = {}
            for i0, glen in groups:
                osb = outp.tile([P, glen, D], f16, tag=f"osb{glen}",
                                name=f"osb{b}_{i0}")
                for q in range(glen):
                    osbs[i0 + q] = (osb, q)

            pss = {}

            def start_tile(i):
                ps = psA.tile([P, D], f32, tag="psa", name=f"ps{b}_{i}")
                pss[i] = ps
                nc.tensor.matmul(
                    ps,
                    lhsT=xta_sb[:, i * P : (i + 1) * P],
                    rhs=wt01_sb,
                    start=True,
                    stop=False,
                )
                nc.tensor.matmul(
                    ps,
                    lhsT=xta_sb[0:C, i * P + 2 : (i + 1) * P + 2],
                    rhs=wtbc_sb[0:C, :],
                    start=False,
                    stop=False,
                )
                if DRAIN[i] == "A":
                    nc.tensor.matmul(
                        ps,
                        lhsT=identh,
                        rhs=pe_sb[:, i, :],
                        start=False,
                        stop=False,
                    )

            def finish_tile(i):
                ps = pss.pop(i)
                nc.tensor.matmul(
                    ps,
                    lhsT=taujp[:, i, :],
                    rhs=wtbc_sb[64:66, :],
                    start=False,
                    stop=True,
                    tile_position=(64, 0),
                )
                osb, q = osbs[i]
                if DRAIN[i] == "A":
                    nc.scalar.copy(osb[:, q, :], ps)
                else:
                    nc.vector.tensor_tensor(
                        osb[:, q, :], ps, pe_sb[:, i, :], op=Alu.add
                    )

            emitted = 0
            done = 0
            for i in range(NT):
                start_tile(i)
                emitted += 1
                if emitted > LAG:
                    finish_tile(done)
                    done += 1
                    # flush any completed output group
                    for gi, (i0, glen) in enumerate(groups):
                        if i0 + glen == done:
                            osb, _ = osbs[i0]
                            dst = out[b, i0 * P : (i0 + glen) * P, :].rearrange(
                                "(q p) d -> p q d", p=P
                            )
                            eng = nc.sync if gi % 2 == 0 else nc.scalar
                            eng.dma_start(dst, osb)
            while done < NT:
                finish_tile(done)
                done += 1
                for gi, (i0, glen) in enumerate(groups):
                    if i0 + glen == done:
                        osb, _ = osbs[i0]
                        dst = out[b, i0 * P : (i0 + glen) * P, :].rearrange(
                            "(q p) d -> p q d", p=P
                        )
                        eng = nc.sync if gi % 2 == 0 else nc.scalar
                        eng.dma_start(dst, osb)


def build_bass():
    import concourse.tile as tile
    from concourse import bacc, mybir

    f32 = mybir.dt.float32
    f16 = mybir.dt.float16

    nc = bacc.Bacc(
        "TRN2",
        target_bir_lowering=False,
        debug=False,
        enable_asserts=False,
        num_devices=NCORES,
    )
    aps = {}
    aps["xin"] = nc.dram_tensor("xin", (BLOC, S, C), f32, kind="ExternalInput").ap()
    aps["xta"] = nc.dram_tensor(
        "xta", (BLOC, P, S + 2), f16, kind="ExternalInput"
    ).ap()
    aps["pe16"] = nc.dram_tensor("pe16", (P, NT, D), f16, kind="ExternalInput").ap()
    aps["wt01"] = nc.dram_tensor("wt01", (P, D), f16, kind="ExternalInput").ap()
    aps["wtbc"] = nc.dram_tensor("wtbc", (66, D), f16, kind="ExternalInput").ap()
    aps["blob16"] = nc.dram_tensor(
        "blob16", (P, B16_W), f16, kind="ExternalInput"
    ).ap()
    aps["blob32"] = nc.dram_tensor(
        "blob32", (P, B32_W), f32, kind="ExternalInput"
    ).ap()
    aps["dtmpl"] = nc.dram_tensor("dtmpl", (6, S), f16, kind="ExternalInput").ap()
    aps["rtmpl"] = nc.dram_tensor("rtmpl", (6, S), f16, kind="ExternalInput").ap()
    aps["nbtt"] = nc.dram_tensor("nbtt", (96, P), f16, kind="ExternalInput").ap()
    aps["out"] = nc.dram_tensor("out", (BLOC, S, D), f16, kind="ExternalOutput").ap()
    aps["js_dram"] = nc.dram_tensor("js_scr", (BLOC, S), f16, kind="Internal").ap()
    aps["spT_d"] = nc.dram_tensor("spT_d", (BLOC, 96, P), f16, kind="Internal").ap()
    aps["tau_dram"] = nc.dram_tensor(
        "tau_scratch", (BLOC, S), f32, kind="Internal"
    ).ap()

    with tile.TileContext(nc) as tc:
        _emit(tc, aps)
    nc.compile()
    return nc


def make_consts():
    position = np.arange(S, dtype=np.float32)[:, None]
    div_term = np.exp(
        np.arange(0, D, 2, dtype=np.float32) * np.float32(-math.log(10000.0) / D)
    ).astype(np.float32)
    ang = (position * div_term).astype(np.float32)
    pe = np.zeros((S, D), dtype=np.float32)
    pe[:, 0::2] = np.sin(ang)
    pe[:, 1::2] = np.cos(ang)

    pp = np.arange(P)
    ii = np.arange(NT)
    uu = np.arange(P)
    jj = np.arange(NT)

    blob16 = np.zeros((P, B16_W), dtype=np.float16)
    blob16[:, B16_IDENT : B16_IDENT + P] = np.eye(P, dtype=np.float16)
    dval = (uu[None, None, :] < pp[:, None, None]) * (
        ii[None, :, None] * P + uu[None, None, :] + 1.0
    )
    blob16[:, B16_DVAL:B16_SVAL] = dval.reshape(P, -1).astype(np.float16)
    blob16[:, B16_SVAL:B16_MASK1] = (uu[None, :] + 1.0).astype(np.float16)
    mask1 = (jj[None, :] < ii[:, None]) * (jj[None, :] + 1.0)
    blob16[:, B16_MASK1:B16_W] = mask1.reshape(1, -1).astype(np.float16)

    blob32 = np.zeros((P, B32_W), dtype=np.float32)
    blob32[:, B32_IDENT : B32_IDENT + P] = np.eye(P, dtype=np.float32)
    blob32[:, B32_T1C : B32_T1C + NT] = (
        ii[None, :] * P + pp[:, None] + 1.0
    ).astype(np.float32)
    kc = np.array(list(range(1, 17)) + [0] + [99] * 15, dtype=np.float32)
    blob32[:, B32_KCOL] = np.concatenate([np.tile(kc, 3), [99] * 32]).astype(
        np.float32
    )[:P]
    return pe, blob16, blob32


def make_shared_inputs(conv_w, tc_w, tc_b):
    pe, blob16, blob32 = make_consts()
    pe_b = (pe + np.asarray(tc_b, np.float32)[None, :]).astype(np.float16)
    # [S, D] -> [P, NT, D] with p = position within tile
    pe_r = np.ascontiguousarray(
        pe_b.reshape(NT, P, D).transpose(1, 0, 2)
    )
    wt = np.transpose(np.asarray(conv_w, np.float32), (2, 1, 0))  # (k, c, d)
    wt01 = np.concatenate([wt[0], wt[1]], axis=0).astype(np.float16)
    w = np.asarray(tc_w, np.float32)[:, 0]
    w_hi = w.astype(np.float16)
    w_lo = (w - w_hi.astype(np.float32)).astype(np.float16)
    wtbc = np.concatenate(
        [wt[2].astype(np.float16), w_hi[None], w_lo[None]], axis=0
    )
    dtmpl = np.zeros((6, S), dtype=np.float16)
    dtmpl[0::2] = 1.0
    rtmpl = np.zeros((6, S), dtype=np.float16)
    rtmpl[1::2] = 1.0
    nbtt = np.zeros((96, P), dtype=np.float16)
    nbtt[16] = -BIG
    nbtt[17::32] = 1.0
    return {
        "pe16": pe_r,
        "wt01": np.ascontiguousarray(wt01),
        "wtbc": np.ascontiguousarray(wtbc),
        "blob16": blob16,
        "blob32": blob32,
        "dtmpl": dtmpl,
        "rtmpl": rtmpl,
        "nbtt": nbtt,
    }


def make_xta(x16):
    bl = x16.shape[0]
    xt = np.transpose(x16, (0, 2, 1))  # (bl, C, S)
    xta = np.zeros((bl, P, S + 2), dtype=np.float16)
    xta[:, 0:C, 1 : S + 1] = xt
    xta[:, 0:C, 0] = xt[:, :, S - 1]
    xta[:, 0:C, S + 1] = xt[:, :, 0]
    xta[:, C : 2 * C, 0:S] = xt
    xta[:, C : 2 * C, S] = xt[:, :, 0]
    return xta


_BUILD_CACHE = {}


def _install_ntff_hook():
    import sys as _sys
    import types

    if "antenv.axon_hooks" in _sys.modules:
        return
    try:
        from trn_agent_boot.trn_boot import _ntff_profile_via_ctypes

        hook = _ntff_profile_via_ctypes("/opt/axon/libaxon_pjrt.so")
        m = types.ModuleType("antenv.axon_hooks")
        m.get_axon_ntff_profile_hook = lambda: hook
        _sys.modules["antenv.axon_hooks"] = m
    except Exception as e:
        print("[kernel] ntff hook install failed:", e)


def kernel(x, conv_w, tc_w, tc_b):
    x = np.ascontiguousarray(np.asarray(x, dtype=np.float32))
    conv_w = np.asarray(conv_w, dtype=np.float32)
    tc_w = np.asarray(tc_w, dtype=np.float32)
    tc_b = np.asarray(tc_b, dtype=np.float32)
    assert x.shape == (B, S, C), x.shape

    from concourse.bass_utils import run_bass_kernel_spmd

    if "nc" not in _BUILD_CACHE:
        _BUILD_CACHE["nc"] = build_bass()
    nc = _BUILD_CACHE["nc"]

    shared = make_shared_inputs(conv_w, tc_w, tc_b)
    x16 = x.astype(np.float16)
    in_maps = []
    for c in range(NCORES):
        m = dict(shared)
        m["xin"] = np.ascontiguousarray(x[c * BLOC : (c + 1) * BLOC])
        m["xta"] = make_xta(x16[c * BLOC : (c + 1) * BLOC])
        in_maps.append(m)

    trace = bool(int(os.environ.get("KERNEL_TRACE", "0")))
    if trace:
        _install_ntff_hook()
    res = run_bass_kernel_spmd(
        nc, in_maps, core_ids=list(range(NCORES)), trace=trace, trace_cores=[0]
    )
    if trace and res.exec_time_ns is not None:
        print(
            f"[kernel] HW exec time: {res.exec_time_ns} ns "
            f"(mean {res.mean_exec_time_ns} ns)"
        )
        kernel.last_exec_time_ns = res.exec_time_ns
        kernel.last_trace = res.instructions_and_trace
    out = np.concatenate([r["out"] for r in res.results], axis=0).astype(np.float32)
    return out


if __name__ == "__main__":
    build_bass()
    print("build ok")
